# revision 1
# baseline (speedup 1.0000x reference)
import jax
import jax.numpy as jnp
import numpy as np
from functools import partial

# nn_DFSGlimpseSingleObjectClassifier — data-parallel over batch on 8 NeuronCores.
# Shapes hardcoded per problem spec: x [128,3,224,224] f32; B=128 -> 16 per core.

WALK = [(0, 1), (1, 3), (3, 1), (1, 4), (4, 1), (1, 0),
        (0, 2), (2, 5), (5, 2), (2, 6), (6, 2), (2, 0)]
G = 15
N_CORES = 8


def _gaussian_glimpse(x, b):
    B, C, H, W = x.shape
    cx = (jnp.tanh(b[:, 0]) + 1.0) * 0.5 * (W - 1)
    cy = (jnp.tanh(b[:, 1]) + 1.0) * 0.5 * (H - 1)
    dx = jax.nn.sigmoid(b[:, 2]) * (W - 1) / (G - 1)
    dy = jax.nn.sigmoid(b[:, 3]) * (H - 1) / (G - 1)
    sx = jnp.exp(b[:, 4]) * dx + 1e-3
    sy = jnp.exp(b[:, 5]) * dy + 1e-3
    offs = jnp.arange(G, dtype=x.dtype) - (G - 1) / 2.0
    mux = cx[:, None] + offs[None, :] * dx[:, None]
    muy = cy[:, None] + offs[None, :] * dy[:, None]
    wr = jnp.arange(W, dtype=x.dtype)
    hr = jnp.arange(H, dtype=x.dtype)
    Fx = jnp.exp(-0.5 * ((wr[None, None, :] - mux[:, :, None]) / sx[:, None, None]) ** 2)
    Fy = jnp.exp(-0.5 * ((hr[None, None, :] - muy[:, :, None]) / sy[:, None, None]) ** 2)
    Fx = Fx / (Fx.sum(-1, keepdims=True) + 1e-8)
    Fy = Fy / (Fy.sum(-1, keepdims=True) + 1e-8)
    # einsum('bgh,bchw,bkw->bcgk') as two batched matmuls (separable filter)
    t = jnp.einsum('bgh,bchw->bcgw', Fy, x)
    return jnp.einsum('bcgw,bkw->bcgk', t, Fx)


def _conv2d(x, w, b):
    y = jax.lax.conv_general_dilated(x, w, (1, 1), 'SAME',
                                     dimension_numbers=('NCHW', 'OIHW', 'NCHW'))
    return y + b[None, :, None, None]


def _adaptive_maxpool_2x2(x):
    s0, s1 = slice(0, 8), slice(7, 15)
    top = jnp.stack([x[..., s0, s0].max((-2, -1)), x[..., s0, s1].max((-2, -1))], -1)
    bot = jnp.stack([x[..., s1, s0].max((-2, -1)), x[..., s1, s1].max((-2, -1))], -1)
    return jnp.stack([top, bot], -2)


def _cnn(g, convs):
    h = g
    n = len(convs)
    for i, (w, b) in enumerate(convs):
        h = _conv2d(h, w, b)
        if i < n - 1:
            h = jax.nn.leaky_relu(h, 0.01)
    h = _adaptive_maxpool_2x2(h)
    return h.reshape(h.shape[0], -1)


def _forward(x, Wb, bb, Wy, by, Wa, ba, convs, Wr, br):
    B = x.shape[0]
    D, P = Wb.shape
    NC = Wy.shape[1]
    N = 7
    h = [jnp.zeros((B, D), x.dtype) for _ in range(N)]
    bx = [jnp.zeros((B, P), x.dtype) for _ in range(N)]
    a = [jnp.zeros((B, 1), x.dtype) for _ in range(N)]
    y = [jnp.zeros((B, NC), x.dtype) for _ in range(N)]

    def update(i, msg):
        h1 = h[i] + msg
        b_new = bx[i] + h1 @ Wb + bb
        y_new = y[i] + h1 @ Wy + by
        a_new = h1 @ Wa + ba
        g = _gaussian_glimpse(x, b_new)
        h_new = h[i] + _cnn(g, convs)
        h[i], bx[i], a[i], y[i] = h_new, b_new, a_new, y_new

    update(0, 0.0)
    for u, v in WALK:
        update(v, h[u])
    return jax.nn.relu((a[0] * h[0]) @ Wr + br)


def _flatten_convs(convs):
    out = []
    for w, b in convs:
        out.append(np.asarray(w, dtype=np.float32))
        out.append(np.asarray(b, dtype=np.float32))
    return out


@partial(jax.pmap, in_axes=(0, None, None, None, None, None, None, None, None,
                            None, None, None, None, None, None, None, None, None))
def _pmapped(x, Wb, bb, Wy, by, Wa, ba, Wr, br,
             cw0, cb0, cw1, cb1, cw2, cb2, cw3, cb3, _dummy):
    convs = ((cw0, cb0), (cw1, cb1), (cw2, cb2), (cw3, cb3))
    return _forward(x, Wb, bb, Wy, by, Wa, ba, convs, Wr, br)


def kernel(x, Wb, bb, Wy, by, Wa, ba, convs, Wr, br):
    x = np.asarray(x, dtype=np.float32)
    B = x.shape[0]
    assert B % N_CORES == 0
    xs = x.reshape(N_CORES, B // N_CORES, *x.shape[1:])
    cf = _flatten_convs(convs)
    out = _pmapped(xs,
                   np.asarray(Wb, np.float32), np.asarray(bb, np.float32),
                   np.asarray(Wy, np.float32), np.asarray(by, np.float32),
                   np.asarray(Wa, np.float32), np.asarray(ba, np.float32),
                   np.asarray(Wr, np.float32), np.asarray(br, np.float32),
                   *cf, np.float32(0.0))
    out = np.asarray(out)
    return out.reshape(B, out.shape[-1])
